# revision 9
# baseline (speedup 1.0000x reference)
"""GCN encoder (conv->BN->ReLU->2 conv heads) on 8 TRN2 NeuronCores.

Sharding: nodes (dst) split 8 ways. Per layer each core computes its shard of
the scaled transform table hs = (h@W)*dis (dis = 1/sqrt(deg)), AllGathers the
full bf16 table, gathers per-edge source rows with dma_gather (int16 indices,
4 source ranges x 4 SWDGE queues), and scatter-adds via one-hot selection
matmuls accumulating in PSUM (node-major). Algebraic folds:
  conv(h,W)[d] = dis[d] * (sum_{e: dst=d} hs[src_e] + hs[d])     (self loop)
  b1 cancels under BatchNorm; heads share one aggregation:
  xm = z@W2+b2, x_ = z@W3+b3 with z = dis*(agg2 + hs2), hs2 = relu(BN(h1))*dis
"""

import sys

sys.path.insert(0, "/opt/trn_rl_repo")

import numpy as np
import ml_dtypes

from concourse import bacc, bass, mybir, tile
from concourse.bass_utils import run_bass_kernel_spmd

bf16 = ml_dtypes.bfloat16

N = 100000
IN = 256
HID = 128
OUT = 64
BN_EPS = 1e-5
NCORES = 8
SH = N // NCORES            # 12500 nodes per core
NB = (SH + 127) // 128      # 98 dst blocks (last has 84 nodes)
CB = 4                      # blocks per gather chunk
NBP = ((NB + CB - 1) // CB) * CB  # 100
NCH = NBP // CB             # 25 chunks
NRANGE = 4
RW = N // NRANGE            # 25000 (< 32768: int16-addressable)
NFULL = SH // 128           # 97 full blocks
REM = SH - NFULL * 128      # 84

# meta layout (f32 [128, MW]): iota(128) | dis_cols(NBP) | b2 | b3 | ones | pcol
MW = 128 + NBP + 4


def _build_nc(tbr):
    gtiles = CB * tbr
    gidx = gtiles * 128
    ncalls = NCH * NRANGE
    idxw = gidx // 16
    dlw = ncalls * gtiles

    nc = bacc.Bacc("TRN2", target_bir_lowering=False, num_devices=NCORES,
                   num_swdge_queues=4)
    f32, b16, i16 = mybir.dt.float32, mybir.dt.bfloat16, mybir.dt.int16

    xT = nc.declare_dram_parameter("xT", [IN, NBP * 128], b16, isOutput=False)
    w1 = nc.declare_dram_parameter("w1", [IN, HID], b16, isOutput=False)
    w2 = nc.declare_dram_parameter("w2", [HID, OUT], b16, isOutput=False)
    w3 = nc.declare_dram_parameter("w3", [HID, OUT], b16, isOutput=False)
    meta = nc.declare_dram_parameter("meta", [128, MW], f32, isOutput=False)
    crow = nc.declare_dram_parameter("crow", [2, 128], f32, isOutput=False)
    idx16 = nc.declare_dram_parameter("idx16", [128, ncalls * idxw], i16,
                                      isOutput=False)
    dlc = nc.declare_dram_parameter("dlc", [128, dlw], b16, isOutput=False)
    metab = nc.declare_dram_parameter("metab", [128, 129], b16, isOutput=False)
    ndl = nc.declare_dram_parameter("ndl", [128, dlw], f32, isOutput=False)
    omu = nc.declare_dram_parameter("omu", [OUT, SH], f32, isOutput=True)
    ols = nc.declare_dram_parameter("ols", [OUT, SH], f32, isOutput=True)

    rg = [list(range(NCORES))]

    def shard_to_rows(dst_dram, src_sb):
        """DMA node-major SBUF blocks [p, b*128+f] -> DRAM rows [b*128+p, f]."""
        nc.sync.dma_start(
            out=dst_dram[0:NFULL * 128, :].rearrange("(b p) f -> p b f", p=128),
            in_=src_sb[:, 0:NFULL * 128].rearrange("p (b f) -> p b f", f=HID),
        )
        nc.sync.dma_start(
            out=dst_dram[NFULL * 128:SH, :],
            in_=src_sb[0:REM, NFULL * 128:NFULL * 128 + HID],
        )

    with tile.TileContext(nc) as tc:
        with (
            tc.tile_pool(name="const", bufs=1) as cp,
            tc.tile_pool(name="dram", bufs=1, space="DRAM") as dp,
            tc.tile_pool(name="big", bufs=1) as bigp,
        ):
            meta_t = cp.tile([128, MW], f32)
            gam_t = cp.tile([1, 128], f32)
            bet_t = cp.tile([1, 128], f32)
            idx_t = cp.tile([128, ncalls * idxw], i16)
            dl_t = cp.tile([128, dlw], b16)
            metab_t = cp.tile([128, 129], b16)
            ndl_t = cp.tile([128, dlw], f32)
            w1_t = cp.tile([128, 2, HID], b16)
            w2_t = cp.tile([HID, OUT], b16)
            w3_t = cp.tile([HID, OUT], b16)
            nc.sync.dma_start(out=meta_t[:], in_=meta[:])
            nc.sync.dma_start(out=gam_t[:], in_=crow[0:1, :])
            nc.sync.dma_start(out=bet_t[:], in_=crow[1:2, :])
            nc.sync.dma_start(out=idx_t[:], in_=idx16[:])
            nc.sync.dma_start(out=dl_t[:], in_=dlc[:])
            nc.sync.dma_start(out=metab_t[:], in_=metab[:])
            nc.sync.dma_start(out=ndl_t[:], in_=ndl[:])
            nc.sync.dma_start(out=w1_t[:],
                              in_=w1[:].rearrange("(k p) n -> p k n", p=128))
            nc.sync.dma_start(out=w2_t[:], in_=w2[:])
            nc.sync.dma_start(out=w3_t[:], in_=w3[:])
            iota = meta_t[:, 0:128]
            iota_b = metab_t[:, 0:128]
            pcol_b = metab_t[:, 128:129]
            dis_cols = meta_t[:, 128:128 + NBP]
            b2c = meta_t[0:OUT, 128 + NBP:129 + NBP]
            b3c = meta_t[0:OUT, 129 + NBP:130 + NBP]
            ones_c = meta_t[:, 130 + NBP:131 + NBP]
            pcol = meta_t[:, 131 + NBP:132 + NBP]

            hs1_sb = bigp.tile([128, NBP * 128], b16)
            h1p_sb = bigp.tile([128, NBP * 128], b16)
            hs2_sb = bigp.tile([128, NBP * 128], b16)
            arep = bigp.tile([128, 128], f32)
            brep = bigp.tile([128, 128], f32)
            ident = bigp.tile([128, 128], b16)
            nc.vector.tensor_tensor(
                out=ident[:], in0=iota_b, in1=pcol_b.to_broadcast([128, 128]),
                op=mybir.AluOpType.is_equal)

            sh1_d = dp.tile([SH, HID], b16)
            sh2_d = dp.tile([SH, HID], b16)
            tab1_d = dp.tile([N, HID], b16)
            tab2_d = dp.tile([N, HID], b16)
            stats_d = dp.tile([2, 128], f32)
            stats2_d = dp.tile([2, 128], f32)

            # ============ transform: hs1 = (x @ W1) * dis ============
            with (
                tc.tile_pool(name="xt", bufs=1) as xp,
                tc.tile_pool(name="tps", bufs=2, space="PSUM") as tpp,
            ):
                xT_t = xp.tile([128, 2, NBP * 128], b16)
                nc.sync.dma_start(
                    out=xT_t[:], in_=xT[:].rearrange("(k p) n -> p k n", p=128))
                for b in range(NBP):
                    ps = tpp.tile([128, HID], f32, space="PSUM", tag="tps")
                    for kk in range(2):
                        nc.tensor.matmul(
                            out=ps[:],
                            lhsT=xT_t[:, kk, b * 128:(b + 1) * 128],
                            rhs=w1_t[:, kk, :],
                            start=(kk == 0), stop=(kk == 1),
                        )
                    nc.vector.tensor_tensor(
                        out=hs1_sb[:, b * 128:(b + 1) * 128], in0=ps[:],
                        in1=dis_cols[:, b:b + 1].to_broadcast([128, 128]),
                        op=mybir.AluOpType.mult,
                    )
            shard_to_rows(sh1_d, hs1_sb)
            nc.gpsimd.collective_compute(
                "AllGather", mybir.AluOpType.bypass, replica_groups=rg,
                ins=[sh1_d[:].opt()], outs=[tab1_d[:].opt()],
            )

            # ============ aggregation pass ============
            def agg_pass(tab_d, hsx_sb, out_cb, stats=None):
                with (
                    tc.tile_pool(name="mb", bufs=2) as mp,
                    tc.tile_pool(name="pb", bufs=4) as pp,
                    tc.tile_pool(name="zps", bufs=2, space="PSUM") as zp,
                    tc.tile_pool(name="sps", bufs=1, space="PSUM") as sp,
                    tc.tile_pool(name="ev", bufs=3) as ep,
                ):
                    st_s = st_q = None
                    if stats is not None:
                        st_s = sp.tile([1, 128], f32, space="PSUM", tag="sts")
                        st_q = sp.tile([1, 128], f32, space="PSUM", tag="stq")
                    for c in range(NCH):
                        mts = []
                        for r in range(NRANGE):
                            q = c * NRANGE + r
                            mt = mp.tile([128, gtiles, 128], b16, tag=f"m{r}")
                            nc.gpsimd.dma_gather(
                                out_ap=mt[:],
                                in_ap=tab_d[r * RW:(r + 1) * RW, :],
                                idxs_ap=idx_t[:, q * idxw:(q + 1) * idxw],
                                num_idxs=gidx, num_idxs_reg=gidx,
                                elem_size=HID,
                                single_packet=False,
                                queue_num=r,
                            )
                            mts.append(mt)
                        for j in range(CB):
                            b = c * CB + j
                            ps = zp.tile([128, 128], f32, space="PSUM", tag="z")
                            nmm = 0
                            for r in range(NRANGE):
                                for t in range(tbr):
                                    col = (c * NRANGE + r) * gtiles + j * tbr + t
                                    pt = pp.tile([128, 128], b16, tag="pt")
                                    if t % 2 == 0:
                                        nc.vector.tensor_tensor(
                                            out=pt[:], in0=iota_b,
                                            in1=dl_t[:, col:col + 1].to_broadcast([128, 128]),
                                            op=mybir.AluOpType.is_equal,
                                        )
                                    else:
                                        pa = pp.tile([128, 128], b16, tag="pa")
                                        nc.scalar.activation(
                                            out=pa[:], in_=iota,
                                            func=mybir.ActivationFunctionType.Abs,
                                            bias=ndl_t[:, col:col + 1], scale=1.0)
                                        nc.scalar.activation(
                                            out=pt[:], in_=pa[:],
                                            func=mybir.ActivationFunctionType.Relu,
                                            bias=1.0, scale=-1.0)
                                    nc.tensor.matmul(
                                        out=ps[:], lhsT=pt[:],
                                        rhs=mts[r][:, j * tbr + t, :],
                                        start=(nmm == 0),
                                        stop=(nmm == NRANGE * tbr - 1),
                                    )
                                    nmm += 1
                            tmp = ep.tile([128, 128], f32, tag="tmp")
                            tmp2 = ep.tile([128, 128], f32, tag="tmp2")
                            nc.vector.tensor_tensor(
                                out=tmp[:], in0=ps[:],
                                in1=hsx_sb[:, b * 128:(b + 1) * 128],
                                op=mybir.AluOpType.add,
                            )
                            nc.vector.tensor_tensor(
                                out=tmp2[:], in0=tmp[:],
                                in1=dis_cols[:, b:b + 1].to_broadcast([128, 128]),
                                op=mybir.AluOpType.mult,
                            )
                            out_cb(b, tmp2, ep)
                            if stats is not None:
                                nc.tensor.matmul(
                                    out=st_s[:], lhsT=ones_c, rhs=tmp2[:],
                                    start=(b == 0), stop=(b == NBP - 1),
                                    skip_group_check=True,
                                )
                                sq = ep.tile([128, 128], f32, tag="sq")
                                nc.scalar.square(out=sq[:], in_=tmp2[:])
                                nc.tensor.matmul(
                                    out=st_q[:], lhsT=ones_c, rhs=sq[:],
                                    start=(b == 0), stop=(b == NBP - 1),
                                    skip_group_check=True,
                                )
                    if stats is not None:
                        ssb = ep.tile([1, 128], f32, tag="ssb")
                        qsb = ep.tile([1, 128], f32, tag="qsb")
                        nc.vector.tensor_copy(out=ssb[:], in_=st_s[:])
                        nc.vector.tensor_copy(out=qsb[:], in_=st_q[:])
                        nc.sync.dma_start(out=stats[0:1, :], in_=ssb[:])
                        nc.sync.dma_start(out=stats[1:2, :], in_=qsb[:])

            # ---- pass 1 ----
            def out1(b, tmp2, ep):
                nc.any.tensor_copy(
                    out=h1p_sb[:, b * 128:(b + 1) * 128], in_=tmp2[:])

            agg_pass(tab1_d, hs1_sb, out1, stats=stats_d)

            # ---- BN ----
            nc.gpsimd.collective_compute(
                "AllReduce", mybir.AluOpType.add, replica_groups=rg,
                ins=[stats_d[:].opt()], outs=[stats2_d[:].opt()],
            )
            with tc.tile_pool(name="bn", bufs=1) as bp:
                st_a = bp.tile([1, 128], f32)
                st_b = bp.tile([1, 128], f32)
                nc.sync.dma_start(out=st_a[:], in_=stats2_d[0:1, :])
                nc.sync.dma_start(out=st_b[:], in_=stats2_d[1:2, :])
                mean = bp.tile([1, 128], f32)
                ex2 = bp.tile([1, 128], f32)
                msq = bp.tile([1, 128], f32)
                var = bp.tile([1, 128], f32)
                std = bp.tile([1, 128], f32)
                inv = bp.tile([1, 128], f32)
                arow = bp.tile([1, 128], f32)
                bm = bp.tile([1, 128], f32)
                brow = bp.tile([1, 128], f32)
                ones_r = bp.tile([1, 128], f32)
                nc.vector.tensor_scalar(
                    out=mean[:], in0=st_a[:], scalar1=1.0 / N, scalar2=None,
                    op0=mybir.AluOpType.mult)
                nc.vector.tensor_scalar(
                    out=ex2[:], in0=st_b[:], scalar1=1.0 / N, scalar2=None,
                    op0=mybir.AluOpType.mult)
                nc.vector.tensor_tensor(
                    out=msq[:], in0=mean[:], in1=mean[:], op=mybir.AluOpType.mult)
                nc.vector.tensor_tensor(
                    out=var[:], in0=ex2[:], in1=msq[:],
                    op=mybir.AluOpType.subtract)
                nc.vector.tensor_scalar(
                    out=var[:], in0=var[:], scalar1=BN_EPS, scalar2=None,
                    op0=mybir.AluOpType.add)
                nc.scalar.activation(
                    out=std[:], in_=var[:],
                    func=mybir.ActivationFunctionType.Sqrt, bias=0.0)
                nc.vector.reciprocal(out=inv[:], in_=std[:])
                nc.vector.tensor_tensor(
                    out=arow[:], in0=gam_t[:], in1=inv[:],
                    op=mybir.AluOpType.mult)
                nc.vector.tensor_tensor(
                    out=bm[:], in0=mean[:], in1=arow[:], op=mybir.AluOpType.mult)
                nc.vector.tensor_tensor(
                    out=brow[:], in0=bet_t[:], in1=bm[:],
                    op=mybir.AluOpType.subtract)
                nc.vector.memset(ones_r[:], 1.0)
                with tc.tile_pool(name="bnps", bufs=1, space="PSUM") as bpp:
                    arep_ps = bpp.tile([128, 128], f32, space="PSUM", tag="ar")
                    brep_ps = bpp.tile([128, 128], f32, space="PSUM", tag="br")
                    nc.tensor.matmul(out=arep_ps[:], lhsT=ones_r[:],
                                     rhs=arow[:], start=True, stop=True)
                    nc.tensor.matmul(out=brep_ps[:], lhsT=ones_r[:],
                                     rhs=brow[:], start=True, stop=True)
                    nc.vector.tensor_copy(out=arep[:], in_=arep_ps[:])
                    nc.vector.tensor_copy(out=brep[:], in_=brep_ps[:])

            # ---- table2 = relu(h1p*A + B) * dis ----
            with tc.tile_pool(name="t2", bufs=3) as t2p:
                for b in range(NBP):
                    u = t2p.tile([128, 128], f32, tag="u")
                    u2 = t2p.tile([128, 128], f32, tag="u2")
                    ur = t2p.tile([128, 128], f32, tag="ur")
                    nc.vector.tensor_tensor(
                        out=u[:], in0=h1p_sb[:, b * 128:(b + 1) * 128],
                        in1=arep[:], op=mybir.AluOpType.mult)
                    nc.vector.tensor_tensor(
                        out=u2[:], in0=u[:], in1=brep[:], op=mybir.AluOpType.add)
                    nc.scalar.activation(
                        out=ur[:], in_=u2[:],
                        func=mybir.ActivationFunctionType.Relu)
                    nc.vector.tensor_tensor(
                        out=hs2_sb[:, b * 128:(b + 1) * 128], in0=ur[:],
                        in1=dis_cols[:, b:b + 1].to_broadcast([128, 128]),
                        op=mybir.AluOpType.mult)
            shard_to_rows(sh2_d, hs2_sb)
            nc.gpsimd.collective_compute(
                "AllGather", mybir.AluOpType.bypass, replica_groups=rg,
                ins=[sh2_d[:].opt()], outs=[tab2_d[:].opt()],
            )

            # ---- pass 2 + heads ----
            with (
                tc.tile_pool(name="hd", bufs=3) as hp,
                tc.tile_pool(name="hps", bufs=1, space="PSUM") as hpp,
            ):
                def out2(b, tmp2, ep):
                    if b >= NB:
                        return
                    zb = ep.tile([128, 128], b16, tag="zb")
                    nc.any.tensor_copy(out=zb[:], in_=tmp2[:])
                    zt_ps = hpp.tile([128, 128], b16, space="PSUM", tag="zt")
                    nc.tensor.transpose(out=zt_ps[:], in_=zb[:],
                                        identity=ident[:])
                    zt = hp.tile([128, 128], b16, tag="ztsb")
                    nc.vector.tensor_copy(out=zt[:], in_=zt_ps[:])
                    lo = b * 128
                    w = min(SH, lo + 128) - lo
                    mu_ps = hpp.tile([OUT, 128], f32, space="PSUM", tag="mu")
                    ls_ps = hpp.tile([OUT, 128], f32, space="PSUM", tag="ls")
                    nc.tensor.matmul(out=mu_ps[:], lhsT=w2_t[:], rhs=zt[:],
                                     start=True, stop=True)
                    nc.tensor.matmul(out=ls_ps[:], lhsT=w3_t[:], rhs=zt[:],
                                     start=True, stop=True)
                    mu_sb = hp.tile([OUT, 128], f32, tag="musb")
                    ls_sb = hp.tile([OUT, 128], f32, tag="lssb")
                    nc.vector.tensor_tensor(
                        out=mu_sb[:], in0=mu_ps[:],
                        in1=b2c.to_broadcast([OUT, 128]), op=mybir.AluOpType.add)
                    nc.vector.tensor_tensor(
                        out=ls_sb[:], in0=ls_ps[:],
                        in1=b3c.to_broadcast([OUT, 128]), op=mybir.AluOpType.add)
                    nc.sync.dma_start(out=omu[:, lo:lo + w], in_=mu_sb[:, :w])
                    nc.sync.dma_start(out=ols[:, lo:lo + w], in_=ls_sb[:, :w])

                agg_pass(tab2_d, hs2_sb, out2, stats=None)

    nc.compile()
    return nc


def _preprocess(x, edge_index, W1, b1, gamma, beta, W2, b2, W3, b3):
    src = np.asarray(edge_index[0], dtype=np.int64)
    dst = np.asarray(edge_index[1], dtype=np.int64)
    E = src.shape[0]
    deg = 1.0 + np.bincount(dst, minlength=N).astype(np.float64)
    dis = (1.0 / np.sqrt(deg)).astype(np.float32)

    core = dst // SH
    blk = (dst % SH) // 128
    dloc = (dst % SH) % 128
    rng = src // RW
    rel = (src % RW).astype(np.int64)

    counts = np.zeros((NCORES, NBP, NRANGE), np.int64)
    np.add.at(counts, (core, blk, rng), 1)
    tbr = int(np.ceil(counts.max() / 128))
    gtiles = CB * tbr
    gidx = gtiles * 128
    idxw = gidx // 16
    ncalls = NCH * NRANGE

    gkey = (core * NBP + blk) * NRANGE + rng
    ngroups = NCORES * NBP * NRANGE
    start = np.zeros(ngroups + 1, np.int64)
    np.cumsum(np.bincount(gkey + 1, minlength=ngroups + 1)[1:], out=start[1:])
    order = np.argsort(gkey, kind="stable")
    krank = np.empty(E, np.int64)
    krank[order] = np.arange(E) - start[gkey[order]]

    q = (blk // CB) * NRANGE + rng
    slot = q * gidx + ((blk % CB) * tbr + krank // 128) * 128 + krank % 128

    tot_slots = ncalls * gidx
    ii = np.arange(gidx)
    in_maps = []
    for c in range(NCORES):
        m = core == c
        idx_flat = np.zeros(tot_slots, np.int16)
        dl_flat = np.full(tot_slots, -1.0, np.float32)
        idx_flat[slot[m]] = rel[m].astype(np.int16)
        dl_flat[slot[m]] = dloc[m].astype(np.float32)

        iv = idx_flat.reshape(ncalls, gidx)
        arr = np.zeros((16, ncalls * idxw), np.int16)
        for qq in range(ncalls):
            arr[ii % 16, qq * idxw + ii // 16] = iv[qq]
        idx16_a = np.tile(arr, (8, 1))

        dlc_a = dl_flat.reshape(ncalls * gtiles, 128).T.copy().astype(bf16)

        base = c * SH
        dcp = np.zeros(NBP * 128, np.float32)
        dcp[:SH] = dis[base:base + SH]
        dis_cols = dcp.reshape(NBP, 128).T

        iota = np.tile(np.arange(128, dtype=np.float32), (128, 1))
        b2col = np.zeros((128, 1), np.float32)
        b2col[:OUT, 0] = np.asarray(b2, np.float32)
        b3col = np.zeros((128, 1), np.float32)
        b3col[:OUT, 0] = np.asarray(b3, np.float32)
        ones_col = np.ones((128, 1), np.float32)
        pcol = np.arange(128, dtype=np.float32).reshape(128, 1)
        meta = np.concatenate(
            [iota, dis_cols, b2col, b3col, ones_col, pcol], axis=1)

        crow_a = np.stack([np.asarray(gamma, np.float32),
                           np.asarray(beta, np.float32)], axis=0)

        xs = np.asarray(x[base:base + SH], np.float32)
        xT_a = np.zeros((IN, NBP * 128), np.float32)
        xT_a[:, :SH] = xs.T
        in_maps.append(dict(
            xT=xT_a.astype(bf16),
            w1=np.asarray(W1, np.float32).astype(bf16),
            w2=np.asarray(W2, np.float32).astype(bf16),
            w3=np.asarray(W3, np.float32).astype(bf16),
            meta=meta.astype(np.float32),
            crow=crow_a.astype(np.float32),
            idx16=idx16_a,
            dlc=dlc_a,
            ndl=(-dl_flat.reshape(ncalls * gtiles, 128).T).astype(np.float32),
            metab=np.concatenate([iota, pcol], axis=1).astype(bf16),
        ))
    return in_maps, tbr


_NC_CACHE = {}


def kernel(**inputs):
    in_maps, tbr = _preprocess(**inputs)
    if tbr not in _NC_CACHE:
        _NC_CACHE[tbr] = _build_nc(tbr)
    nc = _NC_CACHE[tbr]
    res = run_bass_kernel_spmd(nc, in_maps, core_ids=list(range(NCORES)))
    xm = np.concatenate([res.results[c]["omu"].T for c in range(NCORES)], axis=0)
    x_ = np.concatenate([res.results[c]["ols"].T for c in range(NCORES)], axis=0)
    return xm.astype(np.float32), x_.astype(np.float32)


# revision 10
# speedup vs baseline: 1.2600x; 1.2600x over previous
"""GCN encoder (conv->BN->ReLU->2 conv heads) on 8 TRN2 NeuronCores.

Sharding: nodes (dst) split 8 ways. Per layer each core computes its shard of
the scaled transform table hs = (h@W)*dis (dis = 1/sqrt(deg)), AllGathers the
full bf16 table, gathers per-edge source rows with dma_gather (int16 indices,
4 source ranges x 4 SWDGE queues), and scatter-adds via one-hot selection
matmuls accumulating in PSUM (node-major). Algebraic folds:
  conv(h,W)[d] = dis[d] * (sum_{e: dst=d} hs[src_e] + hs[d])     (self loop)
  b1 cancels under BatchNorm; heads share one aggregation:
  xm = z@W2+b2, x_ = z@W3+b3 with z = dis*(agg2 + hs2), hs2 = relu(BN(h1))*dis
"""

import sys

sys.path.insert(0, "/opt/trn_rl_repo")

import numpy as np
import ml_dtypes

from concourse import bacc, bass, mybir, tile
from concourse.bass_utils import run_bass_kernel_spmd

bf16 = ml_dtypes.bfloat16

N = 100000
IN = 256
HID = 128
OUT = 64
BN_EPS = 1e-5
NCORES = 8
SH = N // NCORES            # 12500 nodes per core
NB = (SH + 127) // 128      # 98 dst blocks (last has 84 nodes)
CB = 4                      # blocks per gather chunk
NBP = ((NB + CB - 1) // CB) * CB  # 100
NCH = NBP // CB             # 25 chunks
NRANGE = 4
RW = N // NRANGE            # 25000 (< 32768: int16-addressable)
NFULL = SH // 128           # 97 full blocks
REM = SH - NFULL * 128      # 84

# meta layout (f32 [128, MW]): iota(128) | dis_cols(NBP) | b2 | b3 | ones | pcol
MW = 128 + NBP + 4


def _build_nc(tbr):
    gtiles = CB * tbr
    gidx = gtiles * 128
    ncalls = NCH * NRANGE
    idxw = gidx // 16
    dlw = ncalls * gtiles

    nc = bacc.Bacc("TRN2", target_bir_lowering=False, num_devices=NCORES,
                   num_swdge_queues=4)
    f32, b16, i16 = mybir.dt.float32, mybir.dt.bfloat16, mybir.dt.int16

    xT = nc.declare_dram_parameter("xT", [IN, NBP * 128], b16, isOutput=False)
    w1 = nc.declare_dram_parameter("w1", [IN, HID], b16, isOutput=False)
    w2 = nc.declare_dram_parameter("w2", [HID, OUT], b16, isOutput=False)
    w3 = nc.declare_dram_parameter("w3", [HID, OUT], b16, isOutput=False)
    meta = nc.declare_dram_parameter("meta", [128, MW], f32, isOutput=False)
    crow = nc.declare_dram_parameter("crow", [2, 128], f32, isOutput=False)
    idx16 = nc.declare_dram_parameter("idx16", [128, ncalls * idxw], i16,
                                      isOutput=False)
    dlc = nc.declare_dram_parameter("dlc", [128, dlw], b16, isOutput=False)
    metab = nc.declare_dram_parameter("metab", [128, 129], b16, isOutput=False)
    omu = nc.declare_dram_parameter("omu", [OUT, SH], f32, isOutput=True)
    ols = nc.declare_dram_parameter("ols", [OUT, SH], f32, isOutput=True)

    rg = [list(range(NCORES))]

    def shard_to_rows(dst_dram, src_sb):
        """DMA node-major SBUF blocks [p, b*128+f] -> DRAM rows [b*128+p, f]."""
        nc.sync.dma_start(
            out=dst_dram[0:NFULL * 128, :].rearrange("(b p) f -> p b f", p=128),
            in_=src_sb[:, 0:NFULL * 128].rearrange("p (b f) -> p b f", f=HID),
        )
        nc.sync.dma_start(
            out=dst_dram[NFULL * 128:SH, :],
            in_=src_sb[0:REM, NFULL * 128:NFULL * 128 + HID],
        )

    with tile.TileContext(nc) as tc:
        with (
            tc.tile_pool(name="const", bufs=1) as cp,
            tc.tile_pool(name="dram", bufs=1, space="DRAM") as dp,
            tc.tile_pool(name="big", bufs=1) as bigp,
        ):
            meta_t = cp.tile([128, MW], f32)
            gam_t = cp.tile([1, 128], f32)
            bet_t = cp.tile([1, 128], f32)
            idx_t = cp.tile([128, ncalls * idxw], i16)
            dl_t = cp.tile([128, dlw], b16)
            metab_t = cp.tile([128, 129], b16)
            w1_t = cp.tile([128, 2, HID], b16)
            w2_t = cp.tile([HID, OUT], b16)
            w3_t = cp.tile([HID, OUT], b16)
            nc.sync.dma_start(out=meta_t[:], in_=meta[:])
            nc.sync.dma_start(out=gam_t[:], in_=crow[0:1, :])
            nc.sync.dma_start(out=bet_t[:], in_=crow[1:2, :])
            nc.sync.dma_start(out=idx_t[:], in_=idx16[:])
            nc.sync.dma_start(out=dl_t[:], in_=dlc[:])
            nc.sync.dma_start(out=metab_t[:], in_=metab[:])
            nc.sync.dma_start(out=w1_t[:],
                              in_=w1[:].rearrange("(k p) n -> p k n", p=128))
            nc.sync.dma_start(out=w2_t[:], in_=w2[:])
            nc.sync.dma_start(out=w3_t[:], in_=w3[:])
            iota = meta_t[:, 0:128]
            iota_b = metab_t[:, 0:128]
            pcol_b = metab_t[:, 128:129]
            dis_cols = meta_t[:, 128:128 + NBP]
            b2c = meta_t[0:OUT, 128 + NBP:129 + NBP]
            b3c = meta_t[0:OUT, 129 + NBP:130 + NBP]
            ones_c = meta_t[:, 130 + NBP:131 + NBP]
            pcol = meta_t[:, 131 + NBP:132 + NBP]

            hs1_sb = bigp.tile([128, NBP * 128], b16)
            h1p_sb = bigp.tile([128, NBP * 128], b16)
            hs2_sb = bigp.tile([128, NBP * 128], b16)
            arep = bigp.tile([128, 128], f32)
            brep = bigp.tile([128, 128], f32)
            ident = bigp.tile([128, 128], b16)
            nc.vector.tensor_tensor(
                out=ident[:], in0=iota_b, in1=pcol_b.to_broadcast([128, 128]),
                op=mybir.AluOpType.is_equal)

            sh1_d = dp.tile([SH, HID], b16)
            sh2_d = dp.tile([SH, HID], b16)
            tab1_d = dp.tile([N, HID], b16)
            tab2_d = dp.tile([N, HID], b16)
            stats_d = dp.tile([2, 128], f32)
            stats2_d = dp.tile([2, 128], f32)

            # ============ transform: hs1 = (x @ W1) * dis ============
            with (
                tc.tile_pool(name="xt", bufs=1) as xp,
                tc.tile_pool(name="tps", bufs=2, space="PSUM") as tpp,
            ):
                xT_t = xp.tile([128, 2, NBP * 128], b16)
                nc.sync.dma_start(
                    out=xT_t[:], in_=xT[:].rearrange("(k p) n -> p k n", p=128))
                for b in range(NBP):
                    ps = tpp.tile([128, HID], f32, space="PSUM", tag="tps")
                    for kk in range(2):
                        nc.tensor.matmul(
                            out=ps[:],
                            lhsT=xT_t[:, kk, b * 128:(b + 1) * 128],
                            rhs=w1_t[:, kk, :],
                            start=(kk == 0), stop=(kk == 1),
                        )
                    nc.vector.tensor_tensor(
                        out=hs1_sb[:, b * 128:(b + 1) * 128], in0=ps[:],
                        in1=dis_cols[:, b:b + 1].to_broadcast([128, 128]),
                        op=mybir.AluOpType.mult,
                    )
            shard_to_rows(sh1_d, hs1_sb)
            nc.gpsimd.collective_compute(
                "AllGather", mybir.AluOpType.bypass, replica_groups=rg,
                ins=[sh1_d[:].opt()], outs=[tab1_d[:].opt()],
            )

            # ============ aggregation pass ============
            def agg_pass(tab_d, hsx_sb, out_cb, stats=None):
                with (
                    tc.tile_pool(name="mb", bufs=2) as mp,
                    tc.tile_pool(name="pb", bufs=4) as pp,
                    tc.tile_pool(name="zps", bufs=2, space="PSUM") as zp,
                    tc.tile_pool(name="sps", bufs=1, space="PSUM") as sp,
                    tc.tile_pool(name="ev", bufs=3) as ep,
                ):
                    st_s = st_q = None
                    if stats is not None:
                        st_s = sp.tile([1, 128], f32, space="PSUM", tag="sts")
                        st_q = sp.tile([1, 128], f32, space="PSUM", tag="stq")
                    for c in range(NCH):
                        mts = []
                        for r in range(NRANGE):
                            q = c * NRANGE + r
                            mt = mp.tile([128, gtiles, 128], b16, tag=f"m{r}")
                            nc.gpsimd.dma_gather(
                                out_ap=mt[:],
                                in_ap=tab_d[r * RW:(r + 1) * RW, :],
                                idxs_ap=idx_t[:, q * idxw:(q + 1) * idxw],
                                num_idxs=gidx, num_idxs_reg=gidx,
                                elem_size=HID,
                                single_packet=False,
                                queue_num=r,
                            )
                            mts.append(mt)
                        for j in range(CB):
                            b = c * CB + j
                            ps = zp.tile([128, 128], f32, space="PSUM", tag="z")
                            nmm = 0
                            for r in range(NRANGE):
                                for t in range(tbr):
                                    col = (c * NRANGE + r) * gtiles + j * tbr + t
                                    pt = pp.tile([128, 128], b16, tag="pt")
                                    nc.vector.tensor_tensor(
                                        out=pt[:], in0=iota_b,
                                        in1=dl_t[:, col:col + 1].to_broadcast([128, 128]),
                                        op=mybir.AluOpType.is_equal,
                                    )
                                    nc.tensor.matmul(
                                        out=ps[:], lhsT=pt[:],
                                        rhs=mts[r][:, j * tbr + t, :],
                                        start=(nmm == 0),
                                        stop=(nmm == NRANGE * tbr - 1),
                                    )
                                    nmm += 1
                            tmp = ep.tile([128, 128], f32, tag="tmp")
                            tmp2 = ep.tile([128, 128], f32, tag="tmp2")
                            nc.vector.tensor_tensor(
                                out=tmp[:], in0=ps[:],
                                in1=hsx_sb[:, b * 128:(b + 1) * 128],
                                op=mybir.AluOpType.add,
                            )
                            nc.vector.tensor_tensor(
                                out=tmp2[:], in0=tmp[:],
                                in1=dis_cols[:, b:b + 1].to_broadcast([128, 128]),
                                op=mybir.AluOpType.mult,
                            )
                            out_cb(b, tmp2, ep)
                            if stats is not None:
                                nc.tensor.matmul(
                                    out=st_s[:], lhsT=ones_c, rhs=tmp2[:],
                                    start=(b == 0), stop=(b == NBP - 1),
                                    skip_group_check=True,
                                )
                                sq = ep.tile([128, 128], f32, tag="sq")
                                nc.scalar.square(out=sq[:], in_=tmp2[:])
                                nc.tensor.matmul(
                                    out=st_q[:], lhsT=ones_c, rhs=sq[:],
                                    start=(b == 0), stop=(b == NBP - 1),
                                    skip_group_check=True,
                                )
                    if stats is not None:
                        ssb = ep.tile([1, 128], f32, tag="ssb")
                        qsb = ep.tile([1, 128], f32, tag="qsb")
                        nc.vector.tensor_copy(out=ssb[:], in_=st_s[:])
                        nc.vector.tensor_copy(out=qsb[:], in_=st_q[:])
                        nc.sync.dma_start(out=stats[0:1, :], in_=ssb[:])
                        nc.sync.dma_start(out=stats[1:2, :], in_=qsb[:])

            # ---- pass 1 ----
            def out1(b, tmp2, ep):
                nc.any.tensor_copy(
                    out=h1p_sb[:, b * 128:(b + 1) * 128], in_=tmp2[:])

            agg_pass(tab1_d, hs1_sb, out1, stats=stats_d)

            # ---- BN ----
            nc.gpsimd.collective_compute(
                "AllReduce", mybir.AluOpType.add, replica_groups=rg,
                ins=[stats_d[:].opt()], outs=[stats2_d[:].opt()],
            )
            with tc.tile_pool(name="bn", bufs=1) as bp:
                st_a = bp.tile([1, 128], f32)
                st_b = bp.tile([1, 128], f32)
                nc.sync.dma_start(out=st_a[:], in_=stats2_d[0:1, :])
                nc.sync.dma_start(out=st_b[:], in_=stats2_d[1:2, :])
                mean = bp.tile([1, 128], f32)
                ex2 = bp.tile([1, 128], f32)
                msq = bp.tile([1, 128], f32)
                var = bp.tile([1, 128], f32)
                std = bp.tile([1, 128], f32)
                inv = bp.tile([1, 128], f32)
                arow = bp.tile([1, 128], f32)
                bm = bp.tile([1, 128], f32)
                brow = bp.tile([1, 128], f32)
                ones_r = bp.tile([1, 128], f32)
                nc.vector.tensor_scalar(
                    out=mean[:], in0=st_a[:], scalar1=1.0 / N, scalar2=None,
                    op0=mybir.AluOpType.mult)
                nc.vector.tensor_scalar(
                    out=ex2[:], in0=st_b[:], scalar1=1.0 / N, scalar2=None,
                    op0=mybir.AluOpType.mult)
                nc.vector.tensor_tensor(
                    out=msq[:], in0=mean[:], in1=mean[:], op=mybir.AluOpType.mult)
                nc.vector.tensor_tensor(
                    out=var[:], in0=ex2[:], in1=msq[:],
                    op=mybir.AluOpType.subtract)
                nc.vector.tensor_scalar(
                    out=var[:], in0=var[:], scalar1=BN_EPS, scalar2=None,
                    op0=mybir.AluOpType.add)
                nc.scalar.activation(
                    out=std[:], in_=var[:],
                    func=mybir.ActivationFunctionType.Sqrt, bias=0.0)
                nc.vector.reciprocal(out=inv[:], in_=std[:])
                nc.vector.tensor_tensor(
                    out=arow[:], in0=gam_t[:], in1=inv[:],
                    op=mybir.AluOpType.mult)
                nc.vector.tensor_tensor(
                    out=bm[:], in0=mean[:], in1=arow[:], op=mybir.AluOpType.mult)
                nc.vector.tensor_tensor(
                    out=brow[:], in0=bet_t[:], in1=bm[:],
                    op=mybir.AluOpType.subtract)
                nc.vector.memset(ones_r[:], 1.0)
                with tc.tile_pool(name="bnps", bufs=1, space="PSUM") as bpp:
                    arep_ps = bpp.tile([128, 128], f32, space="PSUM", tag="ar")
                    brep_ps = bpp.tile([128, 128], f32, space="PSUM", tag="br")
                    nc.tensor.matmul(out=arep_ps[:], lhsT=ones_r[:],
                                     rhs=arow[:], start=True, stop=True)
                    nc.tensor.matmul(out=brep_ps[:], lhsT=ones_r[:],
                                     rhs=brow[:], start=True, stop=True)
                    nc.vector.tensor_copy(out=arep[:], in_=arep_ps[:])
                    nc.vector.tensor_copy(out=brep[:], in_=brep_ps[:])

            # ---- table2 = relu(h1p*A + B) * dis ----
            with tc.tile_pool(name="t2", bufs=3) as t2p:
                for b in range(NBP):
                    u = t2p.tile([128, 128], f32, tag="u")
                    u2 = t2p.tile([128, 128], f32, tag="u2")
                    ur = t2p.tile([128, 128], f32, tag="ur")
                    nc.vector.tensor_tensor(
                        out=u[:], in0=h1p_sb[:, b * 128:(b + 1) * 128],
                        in1=arep[:], op=mybir.AluOpType.mult)
                    nc.vector.tensor_tensor(
                        out=u2[:], in0=u[:], in1=brep[:], op=mybir.AluOpType.add)
                    nc.scalar.activation(
                        out=ur[:], in_=u2[:],
                        func=mybir.ActivationFunctionType.Relu)
                    nc.vector.tensor_tensor(
                        out=hs2_sb[:, b * 128:(b + 1) * 128], in0=ur[:],
                        in1=dis_cols[:, b:b + 1].to_broadcast([128, 128]),
                        op=mybir.AluOpType.mult)
            shard_to_rows(sh2_d, hs2_sb)
            nc.gpsimd.collective_compute(
                "AllGather", mybir.AluOpType.bypass, replica_groups=rg,
                ins=[sh2_d[:].opt()], outs=[tab2_d[:].opt()],
            )

            # ---- pass 2 + heads ----
            with (
                tc.tile_pool(name="hd", bufs=3) as hp,
                tc.tile_pool(name="hps", bufs=1, space="PSUM") as hpp,
            ):
                def out2(b, tmp2, ep):
                    if b >= NB:
                        return
                    zb = ep.tile([128, 128], b16, tag="zb")
                    nc.any.tensor_copy(out=zb[:], in_=tmp2[:])
                    zt_ps = hpp.tile([128, 128], b16, space="PSUM", tag="zt")
                    nc.tensor.transpose(out=zt_ps[:], in_=zb[:],
                                        identity=ident[:])
                    zt = hp.tile([128, 128], b16, tag="ztsb")
                    nc.vector.tensor_copy(out=zt[:], in_=zt_ps[:])
                    lo = b * 128
                    w = min(SH, lo + 128) - lo
                    mu_ps = hpp.tile([OUT, 128], f32, space="PSUM", tag="mu")
                    ls_ps = hpp.tile([OUT, 128], f32, space="PSUM", tag="ls")
                    nc.tensor.matmul(out=mu_ps[:], lhsT=w2_t[:], rhs=zt[:],
                                     start=True, stop=True)
                    nc.tensor.matmul(out=ls_ps[:], lhsT=w3_t[:], rhs=zt[:],
                                     start=True, stop=True)
                    mu_sb = hp.tile([OUT, 128], f32, tag="musb")
                    ls_sb = hp.tile([OUT, 128], f32, tag="lssb")
                    nc.vector.tensor_tensor(
                        out=mu_sb[:], in0=mu_ps[:],
                        in1=b2c.to_broadcast([OUT, 128]), op=mybir.AluOpType.add)
                    nc.vector.tensor_tensor(
                        out=ls_sb[:], in0=ls_ps[:],
                        in1=b3c.to_broadcast([OUT, 128]), op=mybir.AluOpType.add)
                    nc.sync.dma_start(out=omu[:, lo:lo + w], in_=mu_sb[:, :w])
                    nc.sync.dma_start(out=ols[:, lo:lo + w], in_=ls_sb[:, :w])

                agg_pass(tab2_d, hs2_sb, out2, stats=None)

    nc.compile()
    return nc


def _preprocess(x, edge_index, W1, b1, gamma, beta, W2, b2, W3, b3):
    src = np.asarray(edge_index[0], dtype=np.int64)
    dst = np.asarray(edge_index[1], dtype=np.int64)
    E = src.shape[0]
    deg = 1.0 + np.bincount(dst, minlength=N).astype(np.float64)
    dis = (1.0 / np.sqrt(deg)).astype(np.float32)

    core = dst // SH
    blk = (dst % SH) // 128
    dloc = (dst % SH) % 128
    rng = src // RW
    rel = (src % RW).astype(np.int64)

    counts = np.zeros((NCORES, NBP, NRANGE), np.int64)
    np.add.at(counts, (core, blk, rng), 1)
    tbr = int(np.ceil(counts.max() / 128))
    gtiles = CB * tbr
    gidx = gtiles * 128
    idxw = gidx // 16
    ncalls = NCH * NRANGE

    gkey = (core * NBP + blk) * NRANGE + rng
    ngroups = NCORES * NBP * NRANGE
    start = np.zeros(ngroups + 1, np.int64)
    np.cumsum(np.bincount(gkey + 1, minlength=ngroups + 1)[1:], out=start[1:])
    order = np.argsort(gkey, kind="stable")
    krank = np.empty(E, np.int64)
    krank[order] = np.arange(E) - start[gkey[order]]

    q = (blk // CB) * NRANGE + rng
    slot = q * gidx + ((blk % CB) * tbr + krank // 128) * 128 + krank % 128

    tot_slots = ncalls * gidx
    ii = np.arange(gidx)
    in_maps = []
    for c in range(NCORES):
        m = core == c
        idx_flat = np.zeros(tot_slots, np.int16)
        dl_flat = np.full(tot_slots, -1.0, np.float32)
        idx_flat[slot[m]] = rel[m].astype(np.int16)
        dl_flat[slot[m]] = dloc[m].astype(np.float32)

        iv = idx_flat.reshape(ncalls, gidx)
        arr = np.zeros((16, ncalls * idxw), np.int16)
        for qq in range(ncalls):
            arr[ii % 16, qq * idxw + ii // 16] = iv[qq]
        idx16_a = np.tile(arr, (8, 1))

        dlc_a = dl_flat.reshape(ncalls * gtiles, 128).T.copy().astype(bf16)

        base = c * SH
        dcp = np.zeros(NBP * 128, np.float32)
        dcp[:SH] = dis[base:base + SH]
        dis_cols = dcp.reshape(NBP, 128).T

        iota = np.tile(np.arange(128, dtype=np.float32), (128, 1))
        b2col = np.zeros((128, 1), np.float32)
        b2col[:OUT, 0] = np.asarray(b2, np.float32)
        b3col = np.zeros((128, 1), np.float32)
        b3col[:OUT, 0] = np.asarray(b3, np.float32)
        ones_col = np.ones((128, 1), np.float32)
        pcol = np.arange(128, dtype=np.float32).reshape(128, 1)
        meta = np.concatenate(
            [iota, dis_cols, b2col, b3col, ones_col, pcol], axis=1)

        crow_a = np.stack([np.asarray(gamma, np.float32),
                           np.asarray(beta, np.float32)], axis=0)

        xs = np.asarray(x[base:base + SH], np.float32)
        xT_a = np.zeros((IN, NBP * 128), np.float32)
        xT_a[:, :SH] = xs.T
        in_maps.append(dict(
            xT=xT_a.astype(bf16),
            w1=np.asarray(W1, np.float32).astype(bf16),
            w2=np.asarray(W2, np.float32).astype(bf16),
            w3=np.asarray(W3, np.float32).astype(bf16),
            meta=meta.astype(np.float32),
            crow=crow_a.astype(np.float32),
            idx16=idx16_a,
            dlc=dlc_a,
            metab=np.concatenate([iota, pcol], axis=1).astype(bf16),
        ))
    return in_maps, tbr


_NC_CACHE = {}


def kernel(**inputs):
    in_maps, tbr = _preprocess(**inputs)
    if tbr not in _NC_CACHE:
        _NC_CACHE[tbr] = _build_nc(tbr)
    nc = _NC_CACHE[tbr]
    res = run_bass_kernel_spmd(nc, in_maps, core_ids=list(range(NCORES)))
    xm = np.concatenate([res.results[c]["omu"].T for c in range(NCORES)], axis=0)
    x_ = np.concatenate([res.results[c]["ols"].T for c in range(NCORES)], axis=0)
    return xm.astype(np.float32), x_.astype(np.float32)


# revision 11
# speedup vs baseline: 1.3308x; 1.0561x over previous
"""GCN encoder (conv->BN->ReLU->2 conv heads) on 8 TRN2 NeuronCores.

Sharding: nodes (dst) split 8 ways. Per layer each core computes its shard of
the scaled transform table hs = (h@W)*dis (dis = 1/sqrt(deg)), AllGathers the
full bf16 table, gathers per-edge source rows with dma_gather (int16 indices,
4 source ranges x 4 SWDGE queues), and scatter-adds via one-hot selection
matmuls accumulating in PSUM (node-major). Algebraic folds:
  conv(h,W)[d] = dis[d] * (sum_{e: dst=d} hs[src_e] + hs[d])     (self loop)
  b1 cancels under BatchNorm; heads share one aggregation:
  xm = z@W2+b2, x_ = z@W3+b3 with z = dis*(agg2 + hs2), hs2 = relu(BN(h1))*dis
"""

import sys

sys.path.insert(0, "/opt/trn_rl_repo")

import numpy as np
import ml_dtypes

from concourse import bacc, bass, mybir, tile
from concourse.bass_utils import run_bass_kernel_spmd

bf16 = ml_dtypes.bfloat16

N = 100000
IN = 256
HID = 128
OUT = 64
BN_EPS = 1e-5
NCORES = 8
SH = N // NCORES            # 12500 nodes per core
NB = (SH + 127) // 128      # 98 dst blocks (last has 84 nodes)
CB = 4                      # blocks per gather chunk
NBP = ((NB + CB - 1) // CB) * CB  # 100
NCH = NBP // CB             # 25 chunks
NRANGE = 4
RW = N // NRANGE            # 25000 (< 32768: int16-addressable)
NFULL = SH // 128           # 97 full blocks
REM = SH - NFULL * 128      # 84

# meta layout (f32 [128, MW]): iota(128) | dis_cols(NBP) | b2 | b3 | ones | pcol
MW = 128 + NBP + 4


def _build_nc(tbr):
    gtiles = CB * tbr
    gidx = gtiles * 128
    ncalls = NCH * NRANGE
    idxw = gidx // 16
    dlw = ncalls * gtiles

    nc = bacc.Bacc("TRN2", target_bir_lowering=False, num_devices=NCORES,
                   num_swdge_queues=4)
    f32, b16, i16 = mybir.dt.float32, mybir.dt.bfloat16, mybir.dt.int16

    xT = nc.declare_dram_parameter("xT", [IN, NBP * 128], b16, isOutput=False)
    w1 = nc.declare_dram_parameter("w1", [IN, HID], b16, isOutput=False)
    w2 = nc.declare_dram_parameter("w2", [HID, OUT], b16, isOutput=False)
    w3 = nc.declare_dram_parameter("w3", [HID, OUT], b16, isOutput=False)
    meta = nc.declare_dram_parameter("meta", [128, MW], f32, isOutput=False)
    crow = nc.declare_dram_parameter("crow", [2, 128], f32, isOutput=False)
    idx16 = nc.declare_dram_parameter("idx16", [128, ncalls * idxw], i16,
                                      isOutput=False)
    dlc = nc.declare_dram_parameter("dlc", [128, dlw], b16, isOutput=False)
    metab = nc.declare_dram_parameter("metab", [128, 129], b16, isOutput=False)
    omu = nc.declare_dram_parameter("omu", [OUT, SH], f32, isOutput=True)
    ols = nc.declare_dram_parameter("ols", [OUT, SH], f32, isOutput=True)

    rg = [list(range(NCORES))]

    def shard_to_rows(dst_dram, src_sb):
        """DMA node-major SBUF blocks [p, b*128+f] -> DRAM rows [b*128+p, f]."""
        nc.sync.dma_start(
            out=dst_dram[0:NFULL * 128, :].rearrange("(b p) f -> p b f", p=128),
            in_=src_sb[:, 0:NFULL * 128].rearrange("p (b f) -> p b f", f=HID),
        )
        nc.sync.dma_start(
            out=dst_dram[NFULL * 128:SH, :],
            in_=src_sb[0:REM, NFULL * 128:NFULL * 128 + HID],
        )

    with tile.TileContext(nc) as tc:
        with (
            tc.tile_pool(name="const", bufs=1) as cp,
            tc.tile_pool(name="dram", bufs=1, space="DRAM") as dp,
            tc.tile_pool(name="big", bufs=1) as bigp,
        ):
            meta_t = cp.tile([128, MW], f32)
            gam_t = cp.tile([1, 128], f32)
            bet_t = cp.tile([1, 128], f32)
            idx_t = cp.tile([128, ncalls * idxw], i16)
            dl_t = cp.tile([128, dlw], b16)
            metab_t = cp.tile([128, 129], b16)
            w1_t = cp.tile([128, 2, HID], b16)
            w2_t = cp.tile([HID, OUT], b16)
            w3_t = cp.tile([HID, OUT], b16)
            nc.sync.dma_start(out=meta_t[:], in_=meta[:])
            nc.sync.dma_start(out=gam_t[:], in_=crow[0:1, :])
            nc.sync.dma_start(out=bet_t[:], in_=crow[1:2, :])
            nc.sync.dma_start(out=idx_t[:], in_=idx16[:])
            nc.sync.dma_start(out=dl_t[:], in_=dlc[:])
            nc.sync.dma_start(out=metab_t[:], in_=metab[:])
            nc.sync.dma_start(out=w1_t[:],
                              in_=w1[:].rearrange("(k p) n -> p k n", p=128))
            nc.sync.dma_start(out=w2_t[:], in_=w2[:])
            nc.sync.dma_start(out=w3_t[:], in_=w3[:])
            iota = meta_t[:, 0:128]
            iota_b = metab_t[:, 0:128]
            pcol_b = metab_t[:, 128:129]
            dis_cols = meta_t[:, 128:128 + NBP]
            b2c = meta_t[0:OUT, 128 + NBP:129 + NBP]
            b3c = meta_t[0:OUT, 129 + NBP:130 + NBP]
            ones_c = meta_t[:, 130 + NBP:131 + NBP]
            pcol = meta_t[:, 131 + NBP:132 + NBP]

            hs1_sb = bigp.tile([128, NBP * 128], b16)
            h1p_sb = bigp.tile([128, NBP * 128], b16)
            hs2_sb = bigp.tile([128, NBP * 128], b16)
            arep = bigp.tile([128, 128], f32)
            brep = bigp.tile([128, 128], f32)
            ident = bigp.tile([128, 128], b16)
            nc.vector.tensor_tensor(
                out=ident[:], in0=iota_b, in1=pcol_b.to_broadcast([128, 128]),
                op=mybir.AluOpType.is_equal)

            sh1_d = dp.tile([SH, HID], b16)
            sh2_d = dp.tile([SH, HID], b16)
            tab1_d = dp.tile([N, HID], b16)
            tab2_d = dp.tile([N, HID], b16)
            stats_d = dp.tile([2, 128], f32)
            stats2_d = dp.tile([2, 128], f32)

            # ============ transform: hs1 = (x @ W1) * dis ============
            with (
                tc.tile_pool(name="xt", bufs=1) as xp,
                tc.tile_pool(name="tps", bufs=2, space="PSUM") as tpp,
            ):
                xT_t = xp.tile([128, 2, NBP * 128], b16)
                nc.sync.dma_start(
                    out=xT_t[:], in_=xT[:].rearrange("(k p) n -> p k n", p=128))
                for b in range(NBP):
                    ps = tpp.tile([128, HID], f32, space="PSUM", tag="tps")
                    for kk in range(2):
                        nc.tensor.matmul(
                            out=ps[:],
                            lhsT=xT_t[:, kk, b * 128:(b + 1) * 128],
                            rhs=w1_t[:, kk, :],
                            start=(kk == 0), stop=(kk == 1),
                        )
                    nc.vector.tensor_tensor(
                        out=hs1_sb[:, b * 128:(b + 1) * 128], in0=ps[:],
                        in1=dis_cols[:, b:b + 1].to_broadcast([128, 128]),
                        op=mybir.AluOpType.mult,
                    )
            shard_to_rows(sh1_d, hs1_sb)
            nc.gpsimd.collective_compute(
                "AllGather", mybir.AluOpType.bypass, replica_groups=rg,
                ins=[sh1_d[:].opt()], outs=[tab1_d[:].opt()],
            )

            # ============ aggregation pass ============
            def agg_pass(tab_d, hsx_sb, out_cb, stats=None):
                with (
                    tc.tile_pool(name="mb", bufs=3) as mp,
                    tc.tile_pool(name="pb", bufs=24) as pp,
                    tc.tile_pool(name="zps", bufs=4, space="PSUM") as zp,
                    tc.tile_pool(name="sps", bufs=1, space="PSUM") as sp,
                    tc.tile_pool(name="ev", bufs=6) as ep,
                ):
                    st_s = st_q = None
                    if stats is not None:
                        st_s = sp.tile([1, 128], f32, space="PSUM", tag="sts")
                        st_q = sp.tile([1, 128], f32, space="PSUM", tag="stq")
                    for c in range(NCH):
                        mts = []
                        for r in range(NRANGE):
                            q = c * NRANGE + r
                            mt = mp.tile([128, gtiles, 128], b16, tag=f"m{r}")
                            nc.gpsimd.dma_gather(
                                out_ap=mt[:],
                                in_ap=tab_d[r * RW:(r + 1) * RW, :],
                                idxs_ap=idx_t[:, q * idxw:(q + 1) * idxw],
                                num_idxs=gidx, num_idxs_reg=gidx,
                                elem_size=HID,
                                single_packet=False,
                                queue_num=r,
                            )
                            mts.append(mt)
                        for j in range(CB):
                            b = c * CB + j
                            ps = zp.tile([128, 128], f32, space="PSUM", tag="z")
                            nmm = 0
                            for r in range(NRANGE):
                                for t in range(tbr):
                                    col = (c * NRANGE + r) * gtiles + j * tbr + t
                                    pt = pp.tile([128, 128], b16, tag="pt")
                                    nc.vector.tensor_tensor(
                                        out=pt[:], in0=iota_b,
                                        in1=dl_t[:, col:col + 1].to_broadcast([128, 128]),
                                        op=mybir.AluOpType.is_equal,
                                    )
                                    nc.tensor.matmul(
                                        out=ps[:], lhsT=pt[:],
                                        rhs=mts[r][:, j * tbr + t, :],
                                        start=(nmm == 0),
                                        stop=(nmm == NRANGE * tbr - 1),
                                    )
                                    nmm += 1
                            tmp = ep.tile([128, 128], f32, tag="tmp")
                            tmp2 = ep.tile([128, 128], f32, tag="tmp2")
                            nc.vector.tensor_tensor(
                                out=tmp[:], in0=ps[:],
                                in1=hsx_sb[:, b * 128:(b + 1) * 128],
                                op=mybir.AluOpType.add,
                            )
                            nc.vector.tensor_tensor(
                                out=tmp2[:], in0=tmp[:],
                                in1=dis_cols[:, b:b + 1].to_broadcast([128, 128]),
                                op=mybir.AluOpType.mult,
                            )
                            out_cb(b, tmp2, ep)
                            if stats is not None:
                                nc.tensor.matmul(
                                    out=st_s[:], lhsT=ones_c, rhs=tmp2[:],
                                    start=(b == 0), stop=(b == NBP - 1),
                                    skip_group_check=True,
                                )
                                sq = ep.tile([128, 128], f32, tag="sq")
                                nc.scalar.square(out=sq[:], in_=tmp2[:])
                                nc.tensor.matmul(
                                    out=st_q[:], lhsT=ones_c, rhs=sq[:],
                                    start=(b == 0), stop=(b == NBP - 1),
                                    skip_group_check=True,
                                )
                    if stats is not None:
                        ssb = ep.tile([1, 128], f32, tag="ssb")
                        qsb = ep.tile([1, 128], f32, tag="qsb")
                        nc.vector.tensor_copy(out=ssb[:], in_=st_s[:])
                        nc.vector.tensor_copy(out=qsb[:], in_=st_q[:])
                        nc.sync.dma_start(out=stats[0:1, :], in_=ssb[:])
                        nc.sync.dma_start(out=stats[1:2, :], in_=qsb[:])

            # ---- pass 1 ----
            def out1(b, tmp2, ep):
                nc.any.tensor_copy(
                    out=h1p_sb[:, b * 128:(b + 1) * 128], in_=tmp2[:])

            agg_pass(tab1_d, hs1_sb, out1, stats=stats_d)

            # ---- BN ----
            nc.gpsimd.collective_compute(
                "AllReduce", mybir.AluOpType.add, replica_groups=rg,
                ins=[stats_d[:].opt()], outs=[stats2_d[:].opt()],
            )
            with tc.tile_pool(name="bn", bufs=1) as bp:
                st_a = bp.tile([1, 128], f32)
                st_b = bp.tile([1, 128], f32)
                nc.sync.dma_start(out=st_a[:], in_=stats2_d[0:1, :])
                nc.sync.dma_start(out=st_b[:], in_=stats2_d[1:2, :])
                mean = bp.tile([1, 128], f32)
                ex2 = bp.tile([1, 128], f32)
                msq = bp.tile([1, 128], f32)
                var = bp.tile([1, 128], f32)
                std = bp.tile([1, 128], f32)
                inv = bp.tile([1, 128], f32)
                arow = bp.tile([1, 128], f32)
                bm = bp.tile([1, 128], f32)
                brow = bp.tile([1, 128], f32)
                ones_r = bp.tile([1, 128], f32)
                nc.vector.tensor_scalar(
                    out=mean[:], in0=st_a[:], scalar1=1.0 / N, scalar2=None,
                    op0=mybir.AluOpType.mult)
                nc.vector.tensor_scalar(
                    out=ex2[:], in0=st_b[:], scalar1=1.0 / N, scalar2=None,
                    op0=mybir.AluOpType.mult)
                nc.vector.tensor_tensor(
                    out=msq[:], in0=mean[:], in1=mean[:], op=mybir.AluOpType.mult)
                nc.vector.tensor_tensor(
                    out=var[:], in0=ex2[:], in1=msq[:],
                    op=mybir.AluOpType.subtract)
                nc.vector.tensor_scalar(
                    out=var[:], in0=var[:], scalar1=BN_EPS, scalar2=None,
                    op0=mybir.AluOpType.add)
                nc.scalar.activation(
                    out=std[:], in_=var[:],
                    func=mybir.ActivationFunctionType.Sqrt, bias=0.0)
                nc.vector.reciprocal(out=inv[:], in_=std[:])
                nc.vector.tensor_tensor(
                    out=arow[:], in0=gam_t[:], in1=inv[:],
                    op=mybir.AluOpType.mult)
                nc.vector.tensor_tensor(
                    out=bm[:], in0=mean[:], in1=arow[:], op=mybir.AluOpType.mult)
                nc.vector.tensor_tensor(
                    out=brow[:], in0=bet_t[:], in1=bm[:],
                    op=mybir.AluOpType.subtract)
                nc.vector.memset(ones_r[:], 1.0)
                with tc.tile_pool(name="bnps", bufs=1, space="PSUM") as bpp:
                    arep_ps = bpp.tile([128, 128], f32, space="PSUM", tag="ar")
                    brep_ps = bpp.tile([128, 128], f32, space="PSUM", tag="br")
                    nc.tensor.matmul(out=arep_ps[:], lhsT=ones_r[:],
                                     rhs=arow[:], start=True, stop=True)
                    nc.tensor.matmul(out=brep_ps[:], lhsT=ones_r[:],
                                     rhs=brow[:], start=True, stop=True)
                    nc.vector.tensor_copy(out=arep[:], in_=arep_ps[:])
                    nc.vector.tensor_copy(out=brep[:], in_=brep_ps[:])

            # ---- table2 = relu(h1p*A + B) * dis ----
            with tc.tile_pool(name="t2", bufs=3) as t2p:
                for b in range(NBP):
                    u = t2p.tile([128, 128], f32, tag="u")
                    u2 = t2p.tile([128, 128], f32, tag="u2")
                    ur = t2p.tile([128, 128], f32, tag="ur")
                    nc.vector.tensor_tensor(
                        out=u[:], in0=h1p_sb[:, b * 128:(b + 1) * 128],
                        in1=arep[:], op=mybir.AluOpType.mult)
                    nc.vector.tensor_tensor(
                        out=u2[:], in0=u[:], in1=brep[:], op=mybir.AluOpType.add)
                    nc.scalar.activation(
                        out=ur[:], in_=u2[:],
                        func=mybir.ActivationFunctionType.Relu)
                    nc.vector.tensor_tensor(
                        out=hs2_sb[:, b * 128:(b + 1) * 128], in0=ur[:],
                        in1=dis_cols[:, b:b + 1].to_broadcast([128, 128]),
                        op=mybir.AluOpType.mult)
            shard_to_rows(sh2_d, hs2_sb)
            nc.gpsimd.collective_compute(
                "AllGather", mybir.AluOpType.bypass, replica_groups=rg,
                ins=[sh2_d[:].opt()], outs=[tab2_d[:].opt()],
            )

            # ---- pass 2 + heads ----
            with (
                tc.tile_pool(name="hd", bufs=3) as hp,
                tc.tile_pool(name="hps", bufs=1, space="PSUM") as hpp,
            ):
                def out2(b, tmp2, ep):
                    if b >= NB:
                        return
                    zb = ep.tile([128, 128], b16, tag="zb")
                    nc.any.tensor_copy(out=zb[:], in_=tmp2[:])
                    zt_ps = hpp.tile([128, 128], b16, space="PSUM", tag="zt")
                    nc.tensor.transpose(out=zt_ps[:], in_=zb[:],
                                        identity=ident[:])
                    zt = hp.tile([128, 128], b16, tag="ztsb")
                    nc.vector.tensor_copy(out=zt[:], in_=zt_ps[:])
                    lo = b * 128
                    w = min(SH, lo + 128) - lo
                    mu_ps = hpp.tile([OUT, 128], f32, space="PSUM", tag="mu")
                    ls_ps = hpp.tile([OUT, 128], f32, space="PSUM", tag="ls")
                    nc.tensor.matmul(out=mu_ps[:], lhsT=w2_t[:], rhs=zt[:],
                                     start=True, stop=True)
                    nc.tensor.matmul(out=ls_ps[:], lhsT=w3_t[:], rhs=zt[:],
                                     start=True, stop=True)
                    mu_sb = hp.tile([OUT, 128], f32, tag="musb")
                    ls_sb = hp.tile([OUT, 128], f32, tag="lssb")
                    nc.vector.tensor_tensor(
                        out=mu_sb[:], in0=mu_ps[:],
                        in1=b2c.to_broadcast([OUT, 128]), op=mybir.AluOpType.add)
                    nc.vector.tensor_tensor(
                        out=ls_sb[:], in0=ls_ps[:],
                        in1=b3c.to_broadcast([OUT, 128]), op=mybir.AluOpType.add)
                    nc.sync.dma_start(out=omu[:, lo:lo + w], in_=mu_sb[:, :w])
                    nc.sync.dma_start(out=ols[:, lo:lo + w], in_=ls_sb[:, :w])

                agg_pass(tab2_d, hs2_sb, out2, stats=None)

    nc.compile()
    return nc


def _preprocess(x, edge_index, W1, b1, gamma, beta, W2, b2, W3, b3):
    src = np.asarray(edge_index[0], dtype=np.int64)
    dst = np.asarray(edge_index[1], dtype=np.int64)
    E = src.shape[0]
    deg = 1.0 + np.bincount(dst, minlength=N).astype(np.float64)
    dis = (1.0 / np.sqrt(deg)).astype(np.float32)

    core = dst // SH
    blk = (dst % SH) // 128
    dloc = (dst % SH) % 128
    rng = src // RW
    rel = (src % RW).astype(np.int64)

    counts = np.zeros((NCORES, NBP, NRANGE), np.int64)
    np.add.at(counts, (core, blk, rng), 1)
    tbr = int(np.ceil(counts.max() / 128))
    gtiles = CB * tbr
    gidx = gtiles * 128
    idxw = gidx // 16
    ncalls = NCH * NRANGE

    gkey = (core * NBP + blk) * NRANGE + rng
    ngroups = NCORES * NBP * NRANGE
    start = np.zeros(ngroups + 1, np.int64)
    np.cumsum(np.bincount(gkey + 1, minlength=ngroups + 1)[1:], out=start[1:])
    order = np.argsort(gkey, kind="stable")
    krank = np.empty(E, np.int64)
    krank[order] = np.arange(E) - start[gkey[order]]

    q = (blk // CB) * NRANGE + rng
    slot = q * gidx + ((blk % CB) * tbr + krank // 128) * 128 + krank % 128

    tot_slots = ncalls * gidx
    ii = np.arange(gidx)
    in_maps = []
    for c in range(NCORES):
        m = core == c
        idx_flat = np.zeros(tot_slots, np.int16)
        dl_flat = np.full(tot_slots, -1.0, np.float32)
        idx_flat[slot[m]] = rel[m].astype(np.int16)
        dl_flat[slot[m]] = dloc[m].astype(np.float32)

        iv = idx_flat.reshape(ncalls, gidx)
        arr = np.zeros((16, ncalls * idxw), np.int16)
        for qq in range(ncalls):
            arr[ii % 16, qq * idxw + ii // 16] = iv[qq]
        idx16_a = np.tile(arr, (8, 1))

        dlc_a = dl_flat.reshape(ncalls * gtiles, 128).T.copy().astype(bf16)

        base = c * SH
        dcp = np.zeros(NBP * 128, np.float32)
        dcp[:SH] = dis[base:base + SH]
        dis_cols = dcp.reshape(NBP, 128).T

        iota = np.tile(np.arange(128, dtype=np.float32), (128, 1))
        b2col = np.zeros((128, 1), np.float32)
        b2col[:OUT, 0] = np.asarray(b2, np.float32)
        b3col = np.zeros((128, 1), np.float32)
        b3col[:OUT, 0] = np.asarray(b3, np.float32)
        ones_col = np.ones((128, 1), np.float32)
        pcol = np.arange(128, dtype=np.float32).reshape(128, 1)
        meta = np.concatenate(
            [iota, dis_cols, b2col, b3col, ones_col, pcol], axis=1)

        crow_a = np.stack([np.asarray(gamma, np.float32),
                           np.asarray(beta, np.float32)], axis=0)

        xs = np.asarray(x[base:base + SH], np.float32)
        xT_a = np.zeros((IN, NBP * 128), np.float32)
        xT_a[:, :SH] = xs.T
        in_maps.append(dict(
            xT=xT_a.astype(bf16),
            w1=np.asarray(W1, np.float32).astype(bf16),
            w2=np.asarray(W2, np.float32).astype(bf16),
            w3=np.asarray(W3, np.float32).astype(bf16),
            meta=meta.astype(np.float32),
            crow=crow_a.astype(np.float32),
            idx16=idx16_a,
            dlc=dlc_a,
            metab=np.concatenate([iota, pcol], axis=1).astype(bf16),
        ))
    return in_maps, tbr


_NC_CACHE = {}


def kernel(**inputs):
    in_maps, tbr = _preprocess(**inputs)
    if tbr not in _NC_CACHE:
        _NC_CACHE[tbr] = _build_nc(tbr)
    nc = _NC_CACHE[tbr]
    res = run_bass_kernel_spmd(nc, in_maps, core_ids=list(range(NCORES)))
    xm = np.concatenate([res.results[c]["omu"].T for c in range(NCORES)], axis=0)
    x_ = np.concatenate([res.results[c]["ols"].T for c in range(NCORES)], axis=0)
    return xm.astype(np.float32), x_.astype(np.float32)
